# revision 7
# baseline (speedup 1.0000x reference)
"""Trainium2 Bass kernel for a 2-conv GNN message-passing layer (CXN).

Computation (reassociated vs the reference, exact up to fp reassociation):
    y0 = relu( (A00 @ relu(x0)) @ W0 )     A00: COO [N0, N0], 800k nnz
    y1 = relu(x1)
    y2 = relu( (A12 @ relu(x1)) @ W1 )     A12: COO [N2, N1], 400k nnz

Distribution over 8 NeuronCores: shard output rows (edge destinations);
each core processes exactly the edges targeting its row range, so the
segment-sum is fully local (no collectives). Gather tables (x0, x1) and
weights are replicated. Edges are destination-sorted per 128-row block,
padded to a uniform chunks-per-block so all cores run one SPMD program.

Per 128-edge chunk: indirect-DMA gather of source rows -> ACT relu ->
DVE builds a val-weighted one-hot S ([128e,128r]) -> PE accumulates
S.T @ relu(G) into the block's PSUM accumulator. Per block: PE transpose,
dense (Z @ W) matmul, ACT relu, DMA out.
"""

import math

import numpy as np

M = 8  # cores
N0, N1, N2 = 50000, 150000, 100000
C = 256
P = 128

_CACHE = {}


# ----------------------------------------------------------------- host prep
def _prep_edges(rows, cols, vals, n_out):
    """Shard edges by destination row across M cores; per (core, 128-row
    block) pad the edge list to a uniform multiple of 128. Returns
    (cols_t, vals_t, rloc_t, nblk, cpb): arrays [M, 128, nblk*cpb] where
    column j = chunk j's 128 edges (one per partition)."""
    rows = np.asarray(rows, dtype=np.int64)
    cols = np.asarray(cols, dtype=np.int64)
    vals = np.asarray(vals, dtype=np.float32)
    nnz = rows.shape[0]
    rpc = n_out // M
    nblk = -(-rpc // P)

    core = rows // rpc
    local = rows - core * rpc
    blk = local // P
    rloc = local - blk * P
    key = core * nblk + blk

    order = np.argsort(key, kind="stable")
    skey = key[order]
    scols = cols[order]
    svals = vals[order]
    srloc = rloc[order]

    ngroups = M * nblk
    cnts = np.bincount(skey, minlength=ngroups)
    cpb = max(1, -(-int(cnts.max()) // P))
    eb = cpb * P

    starts = np.zeros(ngroups, dtype=np.int64)
    starts[1:] = np.cumsum(cnts)[:-1]
    pos = np.arange(nnz, dtype=np.int64) - np.repeat(starts, cnts)
    flat = np.repeat(np.arange(ngroups, dtype=np.int64), cnts) * eb + pos

    pc = np.zeros(ngroups * eb, dtype=np.int32)
    pv = np.zeros(ngroups * eb, dtype=np.float32)
    pr = np.zeros(ngroups * eb, dtype=np.float32)
    pc[flat] = scols.astype(np.int32)
    pv[flat] = svals
    pr[flat] = srloc.astype(np.float32)

    # [M, nblk*cpb, 128] -> [M, 128, nblk*cpb]
    def t(a):
        return np.ascontiguousarray(
            a.reshape(M, nblk * cpb, P).transpose(0, 2, 1)
        )

    return t(pc), t(pv), t(pr), nblk, cpb


# --------------------------------------------------------------- bass kernel
def _build(nblk0, cpb0, nblk2, cpb2, r1_tiles, r1_free, repeat=1):
    import concourse.bacc as bacc
    import concourse.bass as bass
    import concourse.mybir as mybir
    from concourse.masks import make_identity
    from concourse.tile import TileContext

    f32 = mybir.dt.float32
    i32 = mybir.dt.int32
    Relu = mybir.ActivationFunctionType.Relu

    R0 = N0 // M
    R2 = N2 // M
    nch0 = nblk0 * cpb0
    nch2 = nblk2 * cpb2

    nc = bacc.Bacc(None, target_bir_lowering=False)

    x0 = nc.declare_dram_parameter("x0", [N0, C], f32, isOutput=False)
    x1 = nc.declare_dram_parameter("x1", [N1, C], f32, isOutput=False)
    x1s = nc.declare_dram_parameter("x1s", [P * r1_tiles, r1_free], f32, isOutput=False)
    w0 = nc.declare_dram_parameter("w0", [C, C], f32, isOutput=False)
    w1 = nc.declare_dram_parameter("w1", [C, C], f32, isOutput=False)
    c00 = nc.declare_dram_parameter("c00", [P, nch0], i32, isOutput=False)
    v00 = nc.declare_dram_parameter("v00", [P, nch0], f32, isOutput=False)
    r00 = nc.declare_dram_parameter("r00", [P, nch0], f32, isOutput=False)
    c12 = nc.declare_dram_parameter("c12", [P, nch2], i32, isOutput=False)
    v12 = nc.declare_dram_parameter("v12", [P, nch2], f32, isOutput=False)
    r12 = nc.declare_dram_parameter("r12", [P, nch2], f32, isOutput=False)

    y0 = nc.declare_dram_parameter("y0", [R0, C], f32, isOutput=True)
    y1 = nc.declare_dram_parameter("y1", [P * r1_tiles, r1_free], f32, isOutput=True)
    y2 = nc.declare_dram_parameter("y2", [R2, C], f32, isOutput=True)

    with TileContext(nc) as tc:
        with (
            tc.tile_pool(name="const", bufs=1) as cpool,
            tc.tile_pool(name="gather", bufs=12) as gpool,
            tc.tile_pool(name="msg", bufs=12) as mpool,
            tc.tile_pool(name="sel", bufs=12) as spool,
            tc.tile_pool(name="zs", bufs=3) as zpool,
            tc.tile_pool(name="tt", bufs=3) as tpool,
            tc.tile_pool(name="res", bufs=3) as rpool,
            tc.tile_pool(name="x1r", bufs=4) as xpool,
            tc.tile_pool(name="pz", bufs=2, space="PSUM") as pzpool,
            tc.tile_pool(name="pt", bufs=2, space="PSUM") as ptpool,
            tc.tile_pool(name="po", bufs=2, space="PSUM") as popool,
        ):
            ident = cpool.tile([P, P], f32)
            make_identity(nc, ident[:])
            # iota_t[p, f] = f (values 0..127 are exact in f32)
            iota_t = cpool.tile([P, P], f32)
            nc.gpsimd.iota(
                iota_t[:],
                pattern=[[1, P]],
                base=0,
                channel_multiplier=0,
                allow_small_or_imprecise_dtypes=True,
            )

            w0a = cpool.tile([P, C], f32, tag="w0a")
            w0b = cpool.tile([P, C], f32, tag="w0b")
            w1a = cpool.tile([P, C], f32, tag="w1a")
            w1b = cpool.tile([P, C], f32, tag="w1b")
            nc.sync.dma_start(out=w0a[:], in_=w0[0:P, :])
            nc.sync.dma_start(out=w0b[:], in_=w0[P:C, :])
            nc.sync.dma_start(out=w1a[:], in_=w1[0:P, :])
            nc.sync.dma_start(out=w1b[:], in_=w1[P:C, :])

            c00_t = cpool.tile([P, nch0], i32, tag="c00")
            v00_t = cpool.tile([P, nch0], f32, tag="v00")
            r00_t = cpool.tile([P, nch0], f32, tag="r00")
            c12_t = cpool.tile([P, nch2], i32, tag="c12")
            v12_t = cpool.tile([P, nch2], f32, tag="v12")
            r12_t = cpool.tile([P, nch2], f32, tag="r12")
            nc.sync.dma_start(out=c00_t[:], in_=c00[:])
            nc.sync.dma_start(out=v00_t[:], in_=v00[:])
            nc.sync.dma_start(out=r00_t[:], in_=r00[:])
            nc.sync.dma_start(out=c12_t[:], in_=c12[:])
            nc.sync.dma_start(out=v12_t[:], in_=v12[:])
            nc.sync.dma_start(out=r12_t[:], in_=r12[:])

            def conv(table, ct, vt, rt, wa, wb, yout, nblk, cpb, rows_out):
                for b in range(nblk):
                    zp = pzpool.tile([P, C], f32, tag="zp")
                    for cc in range(cpb):
                        j = b * cpb + cc
                        g = gpool.tile([P, C], f32, tag="g")
                        nc.gpsimd.indirect_dma_start(
                            out=g[:],
                            out_offset=None,
                            in_=table[:],
                            in_offset=bass.IndirectOffsetOnAxis(
                                ap=ct[:, j : j + 1], axis=0
                            ),
                        )
                        m = mpool.tile([P, C], f32, tag="m")
                        nc.scalar.activation(out=m[:], in_=g[:], func=Relu)
                        s = spool.tile([P, P], f32, tag="s")
                        nc.vector.tensor_scalar(
                            out=s[:],
                            in0=iota_t[:],
                            scalar1=rt[:, j : j + 1],
                            scalar2=vt[:, j : j + 1],
                            op0=mybir.AluOpType.is_equal,
                            op1=mybir.AluOpType.mult,
                        )
                        nc.tensor.matmul(
                            out=zp[:],
                            lhsT=s[:],
                            rhs=m[:],
                            start=(cc == 0),
                            stop=(cc == cpb - 1),
                        )
                    zs = zpool.tile([P, C], f32, tag="zs")
                    nc.vector.tensor_copy(out=zs[:], in_=zp[:])
                    t0p = ptpool.tile([P, P], f32, tag="t0p")
                    t1p = ptpool.tile([P, P], f32, tag="t1p")
                    nc.tensor.transpose(out=t0p[:], in_=zs[:, 0:P], identity=ident[:])
                    nc.tensor.transpose(out=t1p[:], in_=zs[:, P:C], identity=ident[:])
                    t0 = tpool.tile([P, P], f32, tag="t0")
                    t1 = tpool.tile([P, P], f32, tag="t1")
                    nc.vector.tensor_copy(out=t0[:], in_=t0p[:])
                    nc.vector.tensor_copy(out=t1[:], in_=t1p[:])
                    op = popool.tile([P, C], f32, tag="op")
                    nc.tensor.matmul(out=op[:], lhsT=t0[:], rhs=wa[:], start=True, stop=False)
                    nc.tensor.matmul(out=op[:], lhsT=t1[:], rhs=wb[:], start=False, stop=True)
                    res = rpool.tile([P, C], f32, tag="res")
                    nc.scalar.activation(out=res[:], in_=op[:], func=Relu)
                    rows = min(P, rows_out - b * P)
                    nc.sync.dma_start(
                        out=yout[b * P : b * P + rows, :], in_=res[:rows, :]
                    )

            for _rep in range(repeat):
                conv(x0, c00_t, v00_t, r00_t, w0a, w0b, y0, nblk0, cpb0, R0)
                conv(x1, c12_t, v12_t, r12_t, w1a, w1b, y2, nblk2, cpb2, R2)

                # y1 = relu(x1) on this core's shard, viewed as
                # [r1_tiles*128, r1_free]
                for a in range(r1_tiles):
                    xt = xpool.tile([P, r1_free], f32, tag="xt")
                    nc.sync.dma_start(out=xt[:], in_=x1s[a * P : (a + 1) * P, :])
                    rt_ = xpool.tile([P, r1_free], f32, tag="rt")
                    nc.vector.tensor_scalar_max(out=rt_[:], in0=xt[:], scalar1=0.0)
                    nc.sync.dma_start(out=y1[a * P : (a + 1) * P, :], in_=rt_[:])

    nc.compile()
    return nc


# ------------------------------------------------------------------- runner
def _make_runner(nc, n_cores, replicated):
    import jax
    from jax.experimental.shard_map import shard_map
    from jax.sharding import Mesh, NamedSharding, PartitionSpec

    import concourse.mybir as mybir
    from concourse.bass2jax import (
        _bass_exec_p,
        install_neuronx_cc_hook,
        partition_id_tensor,
    )

    install_neuronx_cc_hook()
    partition_name = nc.partition_id_tensor.name if nc.partition_id_tensor else None

    in_names, out_names, out_avals = [], [], []
    for alloc in nc.m.functions[0].allocations:
        if not isinstance(alloc, mybir.MemoryLocationSet):
            continue
        name = alloc.memorylocations[0].name
        if alloc.kind == "ExternalInput":
            if name != partition_name:
                in_names.append(name)
        elif alloc.kind == "ExternalOutput":
            out_names.append(name)
            out_avals.append(
                jax.core.ShapedArray(
                    tuple(alloc.tensor_shape), mybir.dt.np(alloc.dtype)
                )
            )
    n_params = len(in_names)
    all_in_names = list(in_names) + list(out_names)
    if partition_name is not None:
        all_in_names.append(partition_name)

    def _body(*args):
        operands = list(args)
        if partition_name is not None:
            operands.append(partition_id_tensor())
        outs = _bass_exec_p.bind(
            *operands,
            out_avals=tuple(out_avals),
            in_names=tuple(all_in_names),
            out_names=tuple(out_names),
            lowering_input_output_aliases=(),
            sim_require_finite=True,
            sim_require_nnan=True,
            nc=nc,
        )
        return tuple(outs)

    devices = jax.devices()[:n_cores]
    mesh = Mesh(np.asarray(devices), ("core",))
    in_specs = tuple(
        PartitionSpec(None) if name in replicated else PartitionSpec("core")
        for name in in_names
    ) + (PartitionSpec("core"),) * len(out_names)
    out_specs = (PartitionSpec("core"),) * len(out_names)
    fn = jax.jit(
        shard_map(
            _body, mesh=mesh, in_specs=in_specs, out_specs=out_specs, check_rep=False
        ),
        keep_unused=True,
    )

    def prep_inputs(in_maps):
        args = []
        shard = NamedSharding(mesh, PartitionSpec("core"))
        repl = NamedSharding(mesh, PartitionSpec(None))
        for name in in_names:
            if name in replicated:
                args.append(jax.device_put(in_maps[0][name], repl))
            else:
                cat = np.concatenate([m[name] for m in in_maps], axis=0)
                args.append(jax.device_put(cat, shard))
        for av in out_avals:
            z = np.zeros((n_cores * av.shape[0], *av.shape[1:]), av.dtype)
            args.append(jax.device_put(z, shard))
        return args

    def run(args):
        outs = fn(*args)
        jax.block_until_ready(outs)
        return outs

    def split_outs(outs):
        res = []
        for c in range(n_cores):
            d = {}
            for i, name in enumerate(out_names):
                a = np.asarray(outs[i])
                d[name] = a.reshape(n_cores, *out_avals[i].shape)[c]
            res.append(d)
        return res

    return run, prep_inputs, split_outs


REPLICATED = {"x0", "x1", "w0", "w1"}


def _get_compiled(inputs, repeat=1):
    c0t, v0t, r0t, nblk0, cpb0 = _prep_edges(
        inputs["rows_00"], inputs["cols_00"], inputs["vals_00"], N0
    )
    c2t, v2t, r2t, nblk2, cpb2 = _prep_edges(
        inputs["rows_12"], inputs["cols_12"], inputs["vals_12"], N2
    )
    # y1 shard geometry: per-core N1/M rows of C floats, reshaped to
    # [r1_tiles*128, r1_free]
    r1_elems = (N1 // M) * C  # 4.8M
    r1_free = 1250
    r1_tiles = r1_elems // (P * r1_free)
    assert r1_tiles * P * r1_free == r1_elems

    key = (nblk0, cpb0, nblk2, cpb2, r1_tiles, r1_free, repeat)
    if key not in _CACHE:
        nc = _build(*key[:-1], repeat=repeat)
        _CACHE[key] = (nc, _make_runner(nc, M, REPLICATED))
    nc, (run, prep, split) = _CACHE[key]

    x0 = np.ascontiguousarray(np.asarray(inputs["x_0"], dtype=np.float32))
    x1 = np.ascontiguousarray(np.asarray(inputs["x_1"], dtype=np.float32))
    w0 = np.ascontiguousarray(np.asarray(inputs["W0"], dtype=np.float32))
    w1 = np.ascontiguousarray(np.asarray(inputs["W1"], dtype=np.float32))
    r1 = N1 // M
    in_maps = []
    for k in range(M):
        in_maps.append(
            {
                "x0": x0,
                "x1": x1,
                "x1s": x1[k * r1 : (k + 1) * r1].reshape(P * r1_tiles, r1_free),
                "w0": w0,
                "w1": w1,
                "c00": c0t[k],
                "v00": v0t[k],
                "r00": r0t[k],
                "c12": c2t[k],
                "v12": v2t[k],
                "r12": r2t[k],
            }
        )
    return run, prep, split, in_maps, r1_tiles, r1_free


def kernel(**inputs):
    run, prep, split, in_maps, r1_tiles, r1_free = _get_compiled(inputs)
    args = prep(in_maps)
    outs = split(run(args))
    r1 = N1 // M
    y0 = np.concatenate([outs[k]["y0"] for k in range(M)], axis=0)
    y1 = np.concatenate(
        [outs[k]["y1"].reshape(r1, C) for k in range(M)], axis=0
    )
    y2 = np.concatenate([outs[k]["y2"] for k in range(M)], axis=0)
    return (y0, y1, y2)
